# revision 1
# baseline (speedup 1.0000x reference)
"""GAT message-passing kernel for Trainium2 (8 NeuronCores, Bass/Tile).

Strategy (edge/graph parallelism, per the sharding hint):
  - Host: sort edges by dst, split dst-node space into 8 contiguous ranges with
    ~equal edge counts (one range per core). Per core, pack nodes into "blocks"
    of <=S nodes and <=KC*128 edge slots; edges of a block are padded to KC
    128-edge chunks. All numerical work happens on device; the host only
    rearranges indices (sharding) and reassembles rows (unsharding).
  - Device phase 1 (replicated on all cores): T = [feat @ fc_w | el | er]
    written to a DRAM table (N x 136).
  - Device phase 2 (sharded by edges/dst): per 128-edge chunk, indirect-gather
    T rows by src (feat_src + el) and er by dst; compute
    ex = exp(leaky_relu(el[src] + er[dst])); build a one-hot edge->slot matrix P
    from precomputed slot ids; matmul P^T @ [feat_src * ex | ex] accumulated in
    PSUM per block. Epilogue divides by the per-(node,head) denominator
    (the segment softmax normalization) and streams rows out contiguously.
"""

import math
import os
import numpy as np

# ---------------- problem constants (hardcoded; kernel.py is self-contained) ---
N = 100000
F = 128           # input feature dim (= contraction dim)
H = 4             # heads
D = 32            # dim per head
HD = H * D        # 128
TCOLS = F + 2 * H  # 136 = feat_src(128) + el(4) + er(4)
ML = HD + H       # 132 = msg cols + ex cols
NEG = 0.2
NCORES = 8

# ---------------- device tiling parameters ------------------------------------
S = 32            # node slots per block
KC = 4            # 128-edge chunks per block
CHE = 128         # edges per chunk
BSLOTS = KC * CHE # 512 edge slots per block
SUP_B = 4         # blocks per supertile
SUP_CH = SUP_B * KC
PAD_SEG = 100000  # slot id for padding edges (matches no one-hot column)

# phase-1 layout
NPAD = 100352         # 784 tiles of 128 node rows (>= N)
WCH = 2048            # featT columns loaded per DMA (16 tiles)
G1 = 8                # node tiles per T write


def _pack(src, dst, n_cores, n_nodes, n_edges):
    """Host-side index preprocessing. Returns (edata list, node_map list, B)."""
    order = np.argsort(dst, kind="stable")
    s_src = np.asarray(src, np.int64)[order]
    s_dst = np.asarray(dst, np.int64)[order]
    deg = np.bincount(dst, minlength=n_nodes).astype(np.int64)
    assert deg.max() <= BSLOTS, "node degree exceeds block capacity"
    cum = np.cumsum(deg)
    estart = cum - deg
    bnd = [0]
    for k in range(1, n_cores):
        bnd.append(int(np.searchsorted(cum, n_edges * k / n_cores)))
    bnd.append(n_nodes)

    node_block = np.zeros(n_nodes, np.int64)
    node_slot = np.zeros(n_nodes, np.int64)
    nblocks = []
    for k in range(n_cores):
        nb = 0
        cnt = 0
        slots = 0
        for n in range(bnd[k], bnd[k + 1]):
            d = deg[n]
            if cnt >= S or slots + d > BSLOTS:
                nb += 1
                cnt = 0
                slots = 0
            node_block[n] = nb
            node_slot[n] = cnt
            cnt += 1
            slots += d
        nblocks.append(nb + 1 if bnd[k + 1] > bnd[k] else 0)
    B = max(nblocks)
    B = int(math.ceil(B / SUP_B) * SUP_B)

    edatas = []
    node_maps = []
    for k in range(n_cores):
        lo, hi = bnd[k], bnd[k + 1]
        e_lo = int(estart[lo]) if lo < n_nodes else n_edges
        e_hi = int(estart[hi]) if hi < n_nodes else n_edges
        ksrc = s_src[e_lo:e_hi]
        kdst = s_dst[e_lo:e_hi]
        kblk = node_block[kdst]
        kslot = node_slot[kdst]
        # first sorted-edge index of each block (via its first node)
        nodes = np.arange(lo, hi)
        blk_of_node = node_block[lo:hi]
        nb_k = nblocks[k]
        first_edge = np.zeros(max(nb_k, 1), np.int64)
        ub, ui = np.unique(blk_of_node, return_index=True)
        first_edge[ub] = estart[nodes[ui]]
        pos = np.arange(e_lo, e_hi) - first_edge[kblk]
        assert pos.max(initial=0) < BSLOTS
        c = kblk * KC + pos // CHE
        p = pos % CHE
        # per-supertile layout: [src cols | seg cols | dst cols], each
        # SUP_CH wide and contiguous (indirect-DMA offset APs must be
        # contiguous in the last dim)
        sp_of_c = c // SUP_CH
        i_of_c = c % SUP_CH
        base = sp_of_c * 3 * SUP_CH
        ed = np.zeros((CHE, B * KC * 3), np.int32)
        seg_cols = (np.arange(B * KC * 3)
                    .reshape(-1, 3 * SUP_CH)[:, SUP_CH:2 * SUP_CH].reshape(-1))
        ed[:, seg_cols] = PAD_SEG
        ed[p, base + i_of_c] = ksrc
        ed[p, base + SUP_CH + i_of_c] = kslot
        ed[p, base + 2 * SUP_CH + i_of_c] = kdst
        nm = np.full(B * S, -1, np.int64)
        nm[blk_of_node * S + node_slot[lo:hi]] = nodes
        # per-supertile slot -> node id (for the per-supertile er gather);
        # pad slots point at row 0 (gathered junk is never read back)
        nsup_k = B // SUP_B
        nid = np.zeros((SUP_B * S, nsup_k), np.int32)
        nid[(blk_of_node % SUP_B) * S + node_slot[lo:hi],
            blk_of_node // SUP_B] = nodes
        # block-slot of each edge replicated across S partitions, fp32,
        # laid out [S, nchunks*CHE] so PT one-hots build via tensor_scalar
        segT = np.full((1, B * KC * CHE), float(PAD_SEG), np.float32)
        segT[0, c * CHE + p] = (kslot + S * (kblk % SUP_B)).astype(np.float32)
        segT = np.broadcast_to(segT, (SUP_B * S, B * KC * CHE)).copy()
        edatas.append((ed, nid, segT))
        node_maps.append(nm)
    return edatas, node_maps, B


def _build(B, npad=NPAD, wch=WCH, g1=G1):
    """Build the per-core Bass program (identical across cores)."""
    import concourse.bacc as bacc
    import concourse.tile as tile
    import concourse.mybir as mybir
    from concourse.bass import IndirectOffsetOnAxis

    F32 = mybir.dt.float32
    I32 = mybir.dt.int32
    AOT = mybir.AluOpType

    nsup = B // SUP_B
    n_tiles = npad // 128

    nc = bacc.Bacc("TRN2", target_bir_lowering=False, debug=False)
    featT = nc.dram_tensor("featT", [F, npad], F32, kind="ExternalInput")
    fcw = nc.dram_tensor("fcw", [F, HD], F32, kind="ExternalInput")
    attn = nc.dram_tensor("attn", [1, 2 * HD], F32, kind="ExternalInput")
    edata = nc.dram_tensor("edata", [CHE, B * KC * 3], I32, kind="ExternalInput")
    nid_d = nc.dram_tensor("nid", [SUP_B * S, B // SUP_B], I32, kind="ExternalInput")
    segT_d = nc.dram_tensor("segT", [SUP_B * S, B * KC * CHE], F32, kind="ExternalInput")
    T = nc.dram_tensor("T", [npad, TCOLS], F32, kind="Internal")
    out = nc.dram_tensor("out", [B * S, HD], F32, kind="ExternalOutput")

    with tile.TileContext(nc) as tc:
        with tc.tile_pool(name="const", bufs=1) as const:
            # ---- weight prep: W_aug = [fc_w | W_l | W_r] ----
            w_aug = const.tile([F, TCOLS], F32)
            nc.sync.dma_start(out=w_aug[:, 0:HD], in_=fcw[:, :])
            attn_sb = const.tile([1, 2 * HD], F32)
            nc.sync.dma_start(out=attn_sb[:], in_=attn[:, :])
            ab = const.tile([F, 2 * HD], F32)
            nc.gpsimd.partition_broadcast(ab[:], attn_sb[:])
            tmp = const.tile([F, 2 * HD], F32)
            nc.vector.tensor_tensor(
                out=tmp[:].rearrange("p (t w) -> p t w", t=2),
                in0=w_aug[:, None, 0:HD].broadcast_to([F, 2, HD]),
                in1=ab[:].rearrange("p (t w) -> p t w", t=2),
                op=AOT.mult,
            )
            nc.vector.tensor_reduce(
                w_aug[:, HD:HD + 2 * H].rearrange("p (t h) -> p t h", t=2),
                tmp[:].rearrange("p (t h d) -> p t h d", t=2, h=H),
                mybir.AxisListType.X,
                AOT.add,
            )

            # ---- phase 1: T = [feat @ W_aug] ----
            with tc.tile_pool(name="fp", bufs=3) as fpool, \
                 tc.tile_pool(name="p1ps", bufs=8, space="PSUM") as p1ps, \
                 tc.tile_pool(name="st1", bufs=4) as st1p:
                tpw = wch // 128
                for w in range(npad // wch):
                    fsb = fpool.tile([F, wch], F32)
                    nc.sync.dma_start(out=fsb[:], in_=featT[:, w * wch:(w + 1) * wch])
                    for grp in range(tpw // g1):
                        stg = st1p.tile([F, g1 * TCOLS], F32)
                        for j in range(g1):
                            ps = p1ps.tile([128, TCOLS], F32)
                            col0 = (grp * g1 + j) * 128
                            nc.tensor.matmul(
                                out=ps[:],
                                lhsT=fsb[:, col0:col0 + 128],
                                rhs=w_aug[:],
                                start=True, stop=True,
                            )
                            nc.vector.tensor_copy(
                                out=stg[:, j * TCOLS:(j + 1) * TCOLS], in_=ps[:]
                            )
                        t0 = w * tpw + grp * g1
                        nc.sync.dma_start(
                            out=T[t0 * 128:(t0 + g1) * 128, :].rearrange(
                                "(j p) c -> p j c", j=g1),
                            in_=stg[:].rearrange("p (j c) -> p j c", j=g1),
                        )

            # ---- phase 2: edge processing ----
            iot = const.tile([CHE, S], I32)
            nc.gpsimd.iota(iot[:], pattern=[[1, S]], base=0, channel_multiplier=0)
            iot_col = const.tile([SUP_B * S, 1], I32)
            nc.gpsimd.iota(iot_col[:], pattern=[[0, 1]], base=0,
                           channel_multiplier=1)
            iot_colf = const.tile([SUP_B * S, 1], F32)
            nc.vector.tensor_copy(out=iot_colf[:], in_=iot_col[:])
            nid_sb = const.tile([SUP_B * S, nsup], I32)
            nc.sync.dma_start(out=nid_sb[:], in_=nid_d[:, :])

            with tc.tile_pool(name="ed", bufs=5) as edp, \
                 tc.tile_pool(name="gg", bufs=5) as gp, \
                 tc.tile_pool(name="sgt", bufs=4) as sgtp, \
                 tc.tile_pool(name="ers", bufs=6) as ersp, \
                 tc.tile_pool(name="pp", bufs=3) as ppool, \
                 tc.tile_pool(name="pt", bufs=4) as ptp, \
                 tc.tile_pool(name="mx", bufs=3) as mxp, \
                 tc.tile_pool(name="exu", bufs=3) as exup, \
                 tc.tile_pool(name="rr", bufs=8) as rp, \
                 tc.tile_pool(name="st2", bufs=3) as st2p, \
                 tc.tile_pool(name="p2ps", bufs=6, space="PSUM") as p2ps, \
                 tc.tile_pool(name="erps", bufs=2, space="PSUM") as erps:
                for sp in range(nsup):
                    ed = edp.tile([CHE, SUP_CH * 3], I32)
                    nc.sync.dma_start(
                        out=ed[:],
                        in_=edata[:, sp * SUP_CH * 3:(sp + 1) * SUP_CH * 3])
                    ed_src = ed[:, 0:SUP_CH]
                    ed_seg = ed[:, SUP_CH:2 * SUP_CH]

                    sgt = sgtp.tile([SUP_B * S, SUP_CH * CHE], F32)
                    nc.sync.dma_start(
                        out=sgt[:],
                        in_=segT_d[:, sp * SUP_CH * CHE:(sp + 1) * SUP_CH * CHE])

                    # per-supertile er gather: one index per slot (node id)
                    er_sup = ersp.tile([SUP_B * S, H], F32)
                    nc.gpsimd.indirect_dma_start(
                        out=er_sup[:], out_offset=None,
                        in_=T[:, :],
                        in_offset=IndirectOffsetOnAxis(
                            ap=nid_sb[:, sp:sp + 1], axis=0),
                        element_offset=HD + H,
                    )

                    # HW contract: one index per dest partition-row per
                    # indirect DMA -> one gather per 128-edge chunk.
                    g = gp.tile([CHE, SUP_CH * TCOLS], F32)
                    for i in range(SUP_CH):
                        nc.gpsimd.indirect_dma_start(
                            out=g[:, i * TCOLS:(i + 1) * TCOLS], out_offset=None,
                            in_=T[:, :],
                            in_offset=IndirectOffsetOnAxis(
                                ap=ed_src[:, i:i + 1], axis=0),
                        )
                    gv = g[:].rearrange("p (c w) -> p c w", w=TCOLS)

                    P_t = ppool.tile([CHE, SUP_CH * S], F32)
                    nc.vector.tensor_tensor(
                        out=P_t[:].rearrange("p (c s) -> p c s", s=S),
                        in0=iot[:, None, :].broadcast_to([CHE, SUP_CH, S]),
                        in1=ed_seg[:, :, None].broadcast_to([CHE, SUP_CH, S]),
                        op=AOT.is_equal,
                    )

                    # expand er from slots to edges: PT one-hot + tiny matmul
                    u = exup.tile([CHE, SUP_CH * H], F32, tag="u")
                    for i in range(SUP_CH):
                        pt = ptp.tile([SUP_B * S, CHE], F32)
                        nc.vector.tensor_scalar(
                            out=pt[:], in0=sgt[:, i * CHE:(i + 1) * CHE],
                            scalar1=iot_colf[:, 0:1], scalar2=None,
                            op0=AOT.is_equal)
                        erp_ps = erps.tile([CHE, H], F32)
                        nc.tensor.matmul(
                            out=erp_ps[:], lhsT=pt[:],
                            rhs=er_sup[:],
                            start=True, stop=True)
                        nc.vector.tensor_add(
                            out=u[:, i * H:(i + 1) * H],
                            in0=gv[:, i, HD:HD + H],
                            in1=erp_ps[:],
                        )
                    u2 = exup.tile([CHE, SUP_CH * H], F32, tag="u2")
                    nc.vector.scalar_tensor_tensor(
                        out=u2[:], in0=u[:], scalar=NEG, in1=u[:],
                        op0=AOT.mult, op1=AOT.max)
                    ex = exup.tile([CHE, SUP_CH * H], F32, tag="ex")
                    nc.scalar.activation(
                        out=ex[:], in_=u2[:],
                        func=mybir.ActivationFunctionType.Exp)
                    exv = ex[:].rearrange("p (c h) -> p c h", h=H)

                    mx = mxp.tile([CHE, SUP_CH * ML], F32)
                    mv = mx[:].rearrange("p (c w) -> p c w", w=ML)
                    nc.vector.tensor_copy(out=mv[:, :, HD:HD + H], in_=exv)
                    for h in range(H):
                        nc.vector.tensor_tensor(
                            out=mv[:, :, h * D:(h + 1) * D],
                            in0=gv[:, :, h * D:(h + 1) * D],
                            in1=exv[:, :, h:h + 1].broadcast_to([CHE, SUP_CH, D]),
                            op=AOT.mult,
                        )

                    stg = st2p.tile([S, SUP_B * HD], F32)
                    for j in range(SUP_B):
                        ps = p2ps.tile([S, ML], F32)
                        for cl in range(KC):
                            c = j * KC + cl
                            nc.tensor.matmul(
                                out=ps[:],
                                lhsT=P_t[:, c * S:(c + 1) * S],
                                rhs=mx[:, c * ML:(c + 1) * ML],
                                start=(cl == 0), stop=(cl == KC - 1),
                            )
                        r0 = rp.tile([S, H], F32, tag="r0")
                        nc.vector.tensor_scalar_max(r0[:], ps[:, HD:HD + H], 1e-30)
                        r1 = rp.tile([S, H], F32, tag="r1")
                        nc.vector.reciprocal(r1[:], r0[:])
                        nc.vector.tensor_tensor(
                            out=stg[:, j * HD:(j + 1) * HD].rearrange(
                                "p (h d) -> p h d", h=H),
                            in0=ps[:, 0:HD].rearrange("p (h d) -> p h d", h=H),
                            in1=r1[:, :, None].broadcast_to([S, H, D]),
                            op=AOT.mult,
                        )
                    nc.sync.dma_start(
                        out=out[sp * SUP_B * S:(sp + 1) * SUP_B * S, :].rearrange(
                            "(j p) c -> p j c", j=SUP_B),
                        in_=stg[:].rearrange("p (j c) -> p j c", j=SUP_B),
                    )

    nc.compile()
    return nc


_NC_CACHE = {}
LAST_RESULTS = None


def _get_program(B):
    if B not in _NC_CACHE:
        _NC_CACHE[B] = _build(B)
    return _NC_CACHE[B]


def kernel(feat, fc_w, attn_l, attn_r, src, dst):
    from concourse.bass_utils import run_bass_kernel_spmd

    feat = np.asarray(feat, dtype=np.float32)
    fc_w = np.ascontiguousarray(np.asarray(fc_w, dtype=np.float32))
    attn_l = np.asarray(attn_l, dtype=np.float32)
    attn_r = np.asarray(attn_r, dtype=np.float32)
    src = np.asarray(src).astype(np.int64)
    dst = np.asarray(dst).astype(np.int64)
    n_edges = src.shape[0]

    edatas, node_maps, B = _pack(src, dst, NCORES, N, n_edges)

    featT = np.zeros((F, NPAD), np.float32)
    featT[:, :N] = feat.T
    featT = np.ascontiguousarray(featT)
    attn = np.concatenate(
        [attn_l.reshape(-1), attn_r.reshape(-1)]).reshape(1, 2 * HD)
    attn = np.ascontiguousarray(attn.astype(np.float32))

    nc = _get_program(B)
    in_maps = [
        {"featT": featT, "fcw": fc_w, "attn": attn,
         "edata": edatas[k][0], "nid": edatas[k][1], "segT": edatas[k][2]}
        for k in range(NCORES)
    ]
    res = run_bass_kernel_spmd(nc, in_maps, core_ids=list(range(NCORES)))
    global LAST_RESULTS
    LAST_RESULTS = res

    outf = np.zeros((N, HD), np.float32)
    for k in range(NCORES):
        o = np.asarray(res.results[k]["out"])
        nm = node_maps[k]
        m = nm >= 0
        outf[nm[m]] = o[m]
    return outf



# revision 7
# speedup vs baseline: 2.4306x; 2.4306x over previous
"""GAT message-passing kernel for Trainium2 (8 NeuronCores, Bass/Tile).

Strategy (edge/graph parallelism, per the sharding hint):
  - Host: shard dst nodes into 8 fixed ranges of LOCN=12544. Per core, pack
    dst nodes into supertiles of <=128 node slots; each supertile holds 16
    edge chunks of 128 slots: 4 chunks per src bank (src space split into 4
    banks of 25088 rows so gather indices fit int16 for the SWDGE dma_gather
    instruction). Gather groups of GS=2 supertiles make each per-bank gather
    exactly 1024 indices (the SWDGE descriptor-ring capacity).
  - Device phase 1 (replicated): T2[n] = [feat@fc_w | el | er] as bf16 rows
    of 256 elems (512B stride). Phase 1b computes a per-core er table
    (er_loc) for the LOCAL dst range from a host-sliced feat input, keeping
    the SPMD program uniform while er-gather indices stay dst-local.
  - Device phase 2, per group (32 chunks, 4096 edge slots): 4 batched
    dma_gathers (one per src bank, 1024 rows each, round-robin over 4 SWDGE
    queues) fetch [feat_src|el]; 1 small dma_gather fetches er per NODE SLOT
    (256 rows); er is expanded slot->edge with one-hot matmuls (Pc) into a
    single PSUM bank. Compute ex = exp(leaky_relu(el+er)); build one-hot
    edge->slot P_t; 16 accumulating bf16 matmuls per supertile
    P^T @ [feat*ex | ex] into a [128 slots, 132] PSUM tile; divide by the
    summed ex (segment softmax denominator) and stream out rows.
"""

import numpy as np

# ---------------- problem constants (hardcoded; kernel.py is self-contained) ---
N = 100000
F = 128           # input feature dim (= contraction dim)
H = 4             # heads
D = 32            # dim per head
HD = H * D        # 128
TCOLS = F + 2 * H  # 136 = feat_src(128) + el(4) + er(4)
TROW = 256        # bf16 elems per T2 row (512B)
ML = HD + H       # 132 = msg cols + ex cols
NEG = 0.2
NCORES = 8

# ---------------- device tiling parameters ------------------------------------
NPAD = 100352     # node rows padded (= 8*LOCN = 4*BANK)
LOCN = NPAD // 8  # dst nodes per core = 12544
BANK = NPAD // 4  # src rows per bank = 25088
NB = 4            # src banks
SN = 128          # node slots per supertile
CB = 4            # chunks per bank per supertile
CHE = 128         # edges per chunk
GS = 2            # supertiles per gather group
GCH = NB * GS * CB  # chunks per group = 32
GED = GCH * CHE   # edge slots per group = 4096
SEG_PAD = SN      # seg value for padding edge slots (no one-hot match)
IXC = NB * 64 + 16  # eidx cols per group: 4 banks x 64 + 16 er-slot cols

# phase-1 layout
WCH = 2048        # featT columns per DMA load (16 tiles)
G1 = 8            # node tiles per T2 write


def _wrap16(idx):
    """[n] int -> [128, n//16] int16 in the 16-wrapped, core-replicated layout."""
    n = idx.shape[0]
    w = idx.astype(np.int16).reshape(n // 16, 16).T
    return np.tile(w, (8, 1))


def _pack(src, dst):
    """Host-side index preprocessing.

    Returns (per-core dicts of eidx/segd/segT arrays, node_maps, ngroups).
    """
    import ml_dtypes

    src = np.asarray(src, np.int64)
    dst = np.asarray(dst, np.int64)
    order = np.argsort(dst, kind="stable")
    s_src = src[order]
    s_dst = dst[order]
    core_of = s_dst // LOCN
    cuts = np.searchsorted(core_of, np.arange(NCORES + 1))

    packs = []
    for k in range(NCORES):
        lo = k * LOCN
        e0, e1 = cuts[k], cuts[k + 1]
        ksrc = s_src[e0:e1]
        nloc = s_dst[e0:e1] - lo
        kbank = ksrc // BANK

        degb = np.zeros((LOCN, NB), np.int64)
        np.add.at(degb, (nloc, kbank), 1)

        # greedy supertile packing in dst order
        sup_id = np.zeros(LOCN, np.int64)
        node_slot = np.zeros(LOCN, np.int64)
        cur, cnt_n = 0, 0
        cnt_b = np.zeros(NB, np.int64)
        cap = CB * CHE
        for n in range(LOCN):
            dnb = degb[n]
            if cnt_n >= SN or np.any(cnt_b + dnb > cap):
                cur += 1
                cnt_n = 0
                cnt_b[:] = 0
            sup_id[n] = cur
            node_slot[n] = cnt_n
            cnt_n += 1
            cnt_b += dnb
        packs.append((ksrc, nloc, kbank, sup_id, node_slot, cur + 1))

    nsup_max = max(p[5] for p in packs)
    ngroups = (nsup_max + GS - 1) // GS
    nsup_pad = ngroups * GS
    nchunk = ngroups * GCH

    edatas = []
    node_maps = []
    for k in range(NCORES):
        ksrc, nloc, kbank, sup_id, node_slot, nsup = packs[k]
        esup = sup_id[nloc]
        eslot = node_slot[nloc]
        # position within (sup, bank) group, preserving dst order
        key = esup * NB + kbank
        o = np.argsort(key, kind="stable")
        sk = key[o]
        starts = np.searchsorted(sk, np.arange(nsup * NB))
        cnts = np.diff(np.append(starts, len(sk)))
        pos = np.empty(len(sk), np.int64)
        pos[o] = np.arange(len(sk)) - np.repeat(starts, cnts)
        cb = pos // CHE
        assert cb.max(initial=0) < CB
        p = pos % CHE
        gc = ((esup // GS) * GCH + kbank * (GS * CB)
              + (esup % GS) * CB + cb)

        srcloc = np.zeros((CHE, nchunk), np.int16)
        seg = np.full((CHE, nchunk), float(SEG_PAD), np.float32)
        srcloc[p, gc] = (ksrc - kbank * BANK).astype(np.int16)
        seg[p, gc] = eslot.astype(np.float32)

        # er-slot node ids (dst-local), one per supertile slot
        nid = np.zeros(nsup_pad * SN, np.int16)
        nid[sup_id * SN + node_slot] = np.arange(LOCN).astype(np.int16)

        # eidx per group: [bank0 | bank1 | bank2 | bank3 | er-slots]
        eidx = np.zeros((128, ngroups * IXC), np.int16)
        for g in range(ngroups):
            c0 = g * GCH
            x0 = g * IXC
            for b in range(NB):
                run = srcloc[:, c0 + b * GS * CB:c0 + (b + 1) * GS * CB]
                eidx[:, x0 + b * 64:x0 + (b + 1) * 64] = \
                    _wrap16(run.T.reshape(-1))
            eidx[:, x0 + NB * 64:x0 + IXC] = \
                _wrap16(nid[g * GS * SN:(g + 1) * GS * SN])

        seg_bf = seg.astype(ml_dtypes.bfloat16)
        # transposed seg, replicated across partitions (for the Pc one-hot)
        segT = np.ascontiguousarray(
            np.broadcast_to(seg_bf.T.reshape(1, -1), (128, nchunk * CHE)))

        nm = np.full(nsup_pad * SN, -1, np.int64)
        nm[sup_id * SN + node_slot] = np.arange(LOCN) + k * LOCN
        nm[nm >= N] = -1
        edatas.append({"eidx": eidx, "segd": seg_bf, "segT": segT})
        node_maps.append(nm)
    return edatas, node_maps, ngroups


def _build(ngroups, npad=NPAD, wch=WCH, g1=G1):
    """Build the per-core Bass program (identical across cores)."""
    import concourse.bacc as bacc
    import concourse.tile as tile
    import concourse.mybir as mybir

    F32 = mybir.dt.float32
    BF16 = mybir.dt.bfloat16
    I32 = mybir.dt.int32
    I16 = mybir.dt.int16
    AOT = mybir.AluOpType
    ACT = mybir.ActivationFunctionType

    nchunk = ngroups * GCH

    nc = bacc.Bacc("TRN2", target_bir_lowering=False, debug=False,
                   num_swdge_queues=4)
    featT = nc.dram_tensor("featT", [F, npad], BF16, kind="ExternalInput")
    featL = nc.dram_tensor("featL", [F, LOCN], BF16, kind="ExternalInput")
    fcw = nc.dram_tensor("fcw", [F, HD], F32, kind="ExternalInput")
    attn = nc.dram_tensor("attn", [1, 2 * HD], F32, kind="ExternalInput")
    eidx = nc.dram_tensor("eidx", [128, ngroups * IXC], I16,
                          kind="ExternalInput")
    segd = nc.dram_tensor("segd", [CHE, nchunk], BF16, kind="ExternalInput")
    segTd = nc.dram_tensor("segT", [128, nchunk * CHE], BF16,
                           kind="ExternalInput")
    T2 = nc.dram_tensor("T2", [npad, TROW], BF16, kind="Internal")
    erloc = nc.dram_tensor("erloc", [LOCN, 128], BF16, kind="Internal")
    out = nc.dram_tensor("out", [ngroups * GS * SN, HD], F32,
                         kind="ExternalOutput")

    with tile.TileContext(nc) as tc:
        with tc.tile_pool(name="const", bufs=1) as const:
            # ---- weight prep: W_aug = [fc_w | W_l | W_r] (fp32 -> bf16) ----
            w_aug = const.tile([F, TCOLS], F32)
            nc.sync.dma_start(out=w_aug[:, 0:HD], in_=fcw[:, :])
            attn_sb = const.tile([1, 2 * HD], F32)
            nc.sync.dma_start(out=attn_sb[:], in_=attn[:, :])
            ab = const.tile([F, 2 * HD], F32)
            nc.gpsimd.partition_broadcast(ab[:], attn_sb[:])
            tmp = const.tile([F, 2 * HD], F32)
            nc.vector.tensor_tensor(
                out=tmp[:].rearrange("p (t w) -> p t w", t=2),
                in0=w_aug[:, None, 0:HD].broadcast_to([F, 2, HD]),
                in1=ab[:].rearrange("p (t w) -> p t w", t=2),
                op=AOT.mult,
            )
            nc.vector.tensor_reduce(
                w_aug[:, HD:HD + 2 * H].rearrange("p (t h) -> p t h", t=2),
                tmp[:].rearrange("p (t h d) -> p t h d", t=2, h=H),
                mybir.AxisListType.X,
                AOT.add,
            )
            w_bf = const.tile([F, TCOLS], BF16)
            nc.vector.tensor_copy(out=w_bf[:], in_=w_aug[:])

            # ---- phase 1b: er_loc for the local dst range ----
            with tc.tile_pool(name="p1b", bufs=1) as p1b, \
                 tc.tile_pool(name="p1bps", bufs=8, space="PSUM") as p1bps:
                fl = p1b.tile([F, LOCN], BF16)
                nc.sync.dma_start(out=fl[:], in_=featL[:, :])
                ntl = LOCN // 128
                erst = p1b.tile([128, ntl * H], BF16)
                for j in range(ntl):
                    ps = p1bps.tile([128, H], F32)
                    nc.tensor.matmul(
                        out=ps[:], lhsT=fl[:, j * 128:(j + 1) * 128],
                        rhs=w_bf[:, HD + H:HD + 2 * H],
                        start=True, stop=True)
                    nc.vector.tensor_copy(
                        out=erst[:, j * H:(j + 1) * H], in_=ps[:])
                nc.sync.dma_start(
                    out=erloc[:, 0:H].rearrange("(j p) c -> p j c", p=128),
                    in_=erst[:].rearrange("p (j c) -> p j c", c=H))

            # ---- phase 1: T2 = [feat @ W_aug] in bf16 256-elem rows ----
            with tc.tile_pool(name="fp", bufs=3) as fpool, \
                 tc.tile_pool(name="p1ps", bufs=8, space="PSUM") as p1ps, \
                 tc.tile_pool(name="st1", bufs=4) as st1p:
                tpw = wch // 128
                for w in range(npad // wch):
                    fsb = fpool.tile([F, wch], BF16)
                    nc.sync.dma_start(
                        out=fsb[:], in_=featT[:, w * wch:(w + 1) * wch])
                    for grp in range(tpw // g1):
                        stg = st1p.tile([F, g1 * TROW], BF16)
                        for j in range(g1):
                            ps = p1ps.tile([128, TCOLS], F32)
                            col0 = (grp * g1 + j) * 128
                            nc.tensor.matmul(
                                out=ps[:],
                                lhsT=fsb[:, col0:col0 + 128],
                                rhs=w_bf[:],
                                start=True, stop=True,
                            )
                            nc.vector.tensor_copy(
                                out=stg[:, j * TROW:j * TROW + TCOLS],
                                in_=ps[:])
                        t0 = w * tpw + grp * g1
                        nc.sync.dma_start(
                            out=T2[t0 * 128:(t0 + g1) * 128, :].rearrange(
                                "(j p) c -> p j c", j=g1),
                            in_=stg[:].rearrange("p (j c) -> p j c", j=g1),
                        )

            # ---- phase 2: edge processing ----
            ioti = const.tile([CHE, SN], I32)
            nc.gpsimd.iota(ioti[:], pattern=[[1, SN]], base=0,
                           channel_multiplier=0)
            iot = const.tile([CHE, SN], BF16)
            nc.vector.tensor_copy(out=iot[:], in_=ioti[:])
            iotci = const.tile([128, 1], I32)
            nc.gpsimd.iota(iotci[:], pattern=[[0, 1]], base=0,
                           channel_multiplier=1)
            iotc = const.tile([128, 1], BF16)
            nc.vector.tensor_copy(out=iotc[:], in_=iotci[:])

            with tc.tile_pool(name="ix", bufs=3) as ixp, \
                 tc.tile_pool(name="sg", bufs=3) as sgp, \
                 tc.tile_pool(name="sgt", bufs=3) as sgtp, \
                 tc.tile_pool(name="gg", bufs=3) as gp, \
                 tc.tile_pool(name="ee", bufs=3) as ep, \
                 tc.tile_pool(name="pc", bufs=2) as pcp, \
                 tc.tile_pool(name="pp", bufs=2) as ppool, \
                 tc.tile_pool(name="ux", bufs=2) as uxp, \
                 tc.tile_pool(name="mm", bufs=2) as mxp, \
                 tc.tile_pool(name="rr", bufs=8) as rp, \
                 tc.tile_pool(name="so", bufs=4) as sop, \
                 tc.tile_pool(name="erps", bufs=2, space="PSUM") as erpsp, \
                 tc.tile_pool(name="p2ps", bufs=5, space="PSUM") as p2ps:
                for g in range(ngroups):
                    ix = ixp.tile([128, IXC], I16)
                    nc.sync.dma_start(
                        out=ix[:], in_=eidx[:, g * IXC:(g + 1) * IXC])
                    sg = sgp.tile([CHE, GCH], BF16)
                    nc.sync.dma_start(
                        out=sg[:], in_=segd[:, g * GCH:(g + 1) * GCH])
                    sgt = sgtp.tile([128, GED], BF16)
                    nc.sync.dma_start(
                        out=sgt[:], in_=segTd[:, g * GED:(g + 1) * GED])

                    gt = gp.tile([CHE, GCH * TROW], BF16)
                    gv = gt[:].rearrange("p (c e) -> p c e", e=TROW)
                    for b in range(NB):
                        nc.gpsimd.dma_gather(
                            gv[:, b * GS * CB:(b + 1) * GS * CB, :],
                            T2[b * BANK:(b + 1) * BANK, :],
                            ix[:, b * 64:(b + 1) * 64],
                            GS * CB * CHE, GS * CB * CHE, TROW,
                            queue_num=b)

                    ers = ep.tile([CHE, GS * 128], BF16)
                    nc.gpsimd.dma_gather(
                        ers[:].rearrange("p (c e) -> p c e", e=128),
                        erloc[:, :], ix[:, NB * 64:IXC],
                        GS * SN, GS * SN, 128, queue_num=g % 4)
                    erv = ers[:].rearrange("p (c e) -> p c e", e=128)

                    # er expansion: slot -> edge via Pc one-hots
                    Pc = pcp.tile([128, GED], BF16)
                    nc.vector.tensor_tensor(
                        out=Pc[:].rearrange("p (c e) -> p c e", e=CHE),
                        in0=iotc[:, 0:1, None].broadcast_to([128, GCH, CHE]),
                        in1=sgt[:].rearrange("p (c e) -> p c e", e=CHE),
                        op=AOT.is_equal,
                    )
                    erx_ps = erpsp.tile([CHE, GCH * H], F32)
                    for c in range(GCH):
                        s = (c % (GS * CB)) // CB
                        nc.tensor.matmul(
                            out=erx_ps[:, c * H:(c + 1) * H],
                            lhsT=Pc[:, c * CHE:(c + 1) * CHE],
                            rhs=erv[:, s, 0:H],
                            start=True, stop=True,
                        )
                    erx = ep.tile([CHE, GCH * H], BF16, tag="erx")
                    nc.vector.tensor_copy(out=erx[:], in_=erx_ps[:])

                    u = uxp.tile([CHE, GCH * H], F32, tag="u")
                    nc.vector.tensor_tensor(
                        out=u[:].rearrange("p (c h) -> p c h", h=H),
                        in0=gv[:, :, HD:HD + H],
                        in1=erx[:].rearrange("p (c h) -> p c h", h=H),
                        op=AOT.add,
                    )
                    u2 = uxp.tile([CHE, GCH * H], F32, tag="u2")
                    nc.vector.scalar_tensor_tensor(
                        out=u2[:], in0=u[:], scalar=NEG, in1=u[:],
                        op0=AOT.mult, op1=AOT.max)
                    ex = uxp.tile([CHE, GCH * H], BF16, tag="ex")
                    nc.scalar.activation(out=ex[:], in_=u2[:], func=ACT.Exp)
                    exv = ex[:].rearrange("p (c h) -> p c h", h=H)

                    P_t = ppool.tile([CHE, GCH * SN], BF16)
                    nc.vector.tensor_tensor(
                        out=P_t[:].rearrange("p (c s) -> p c s", s=SN),
                        in0=iot[:, None, :].broadcast_to([CHE, GCH, SN]),
                        in1=sg[:, :, None].broadcast_to([CHE, GCH, SN]),
                        op=AOT.is_equal,
                    )

                    mx = mxp.tile([CHE, GCH * ML], BF16)
                    mv = mx[:].rearrange("p (c w) -> p c w", w=ML)
                    nc.vector.tensor_copy(out=mv[:, :, HD:HD + H], in_=exv)
                    nc.vector.tensor_tensor(
                        out=mv[:, :, 0:HD].rearrange(
                            "p c (h d) -> p c h d", h=H),
                        in0=gv[:, :, 0:HD].rearrange(
                            "p c (h d) -> p c h d", h=H),
                        in1=exv[:, :, :, None].broadcast_to([CHE, GCH, H, D]),
                        op=AOT.mult,
                    )

                    for s in range(GS):
                        ps = p2ps.tile([SN, ML], F32)
                        for b in range(NB):
                            for i in range(CB):
                                c = b * GS * CB + s * CB + i
                                nc.tensor.matmul(
                                    out=ps[:],
                                    lhsT=P_t[:, c * SN:(c + 1) * SN],
                                    rhs=mx[:, c * ML:(c + 1) * ML],
                                    start=(b == 0 and i == 0),
                                    stop=(b == NB - 1 and i == CB - 1),
                                )
                        r0 = rp.tile([SN, H], F32, tag="r0")
                        nc.vector.tensor_scalar_max(r0[:], ps[:, HD:HD + H],
                                                    1e-30)
                        r1 = rp.tile([SN, H], F32, tag="r1")
                        nc.vector.reciprocal(r1[:], r0[:])
                        stg = sop.tile([SN, HD], F32)
                        nc.vector.tensor_tensor(
                            out=stg[:].rearrange("p (h d) -> p h d", h=H),
                            in0=ps[:, 0:HD].rearrange("p (h d) -> p h d", h=H),
                            in1=r1[:, :, None].broadcast_to([SN, H, D]),
                            op=AOT.mult,
                        )
                        nc.sync.dma_start(
                            out=out[(g * GS + s) * SN:(g * GS + s + 1) * SN, :],
                            in_=stg[:])

    nc.compile()
    return nc


_NC_CACHE = {}
LAST_RESULTS = None


def _get_program(ngroups):
    if ngroups not in _NC_CACHE:
        _NC_CACHE[ngroups] = _build(ngroups)
    return _NC_CACHE[ngroups]


def kernel(feat, fc_w, attn_l, attn_r, src, dst):
    import ml_dtypes
    from concourse.bass_utils import run_bass_kernel_spmd

    feat = np.asarray(feat, dtype=np.float32)
    fc_w = np.ascontiguousarray(np.asarray(fc_w, dtype=np.float32))
    attn_l = np.asarray(attn_l, dtype=np.float32)
    attn_r = np.asarray(attn_r, dtype=np.float32)
    src = np.asarray(src).astype(np.int64)
    dst = np.asarray(dst).astype(np.int64)

    edatas, node_maps, ngroups = _pack(src, dst)

    featT = np.zeros((F, NPAD), np.float32)
    featT[:, :N] = feat.T
    featT_bf = featT.astype(ml_dtypes.bfloat16)
    attn = np.concatenate(
        [attn_l.reshape(-1), attn_r.reshape(-1)]).reshape(1, 2 * HD)
    attn = np.ascontiguousarray(attn.astype(np.float32))

    nc = _get_program(ngroups)
    in_maps = [
        {"featT": featT_bf,
         "featL": np.ascontiguousarray(featT_bf[:, k * LOCN:(k + 1) * LOCN]),
         "fcw": fc_w, "attn": attn,
         "eidx": edatas[k]["eidx"], "segd": edatas[k]["segd"],
         "segT": edatas[k]["segT"]}
        for k in range(NCORES)
    ]
    res = run_bass_kernel_spmd(nc, in_maps, core_ids=list(range(NCORES)))
    global LAST_RESULTS
    LAST_RESULTS = res

    outf = np.zeros((N, HD), np.float32)
    for k in range(NCORES):
        o = np.asarray(res.results[k]["out"])
        nm = node_maps[k]
        m = nm >= 0
        outf[nm[m]] = o[m]
    return outf
